# revision 1
# baseline (speedup 1.0000x reference)
"""Bayesian dense MoE (top-2 of 8 experts) on 8 Trainium2 NeuronCores.

Math (per reference):
    logits = x @ gk + gb                      [B, E]
    gw     = renorm-top2(softmax(logits))     [B, E]   (softmax denom cancels)
    se     = softplus(rho) * eps              [U, E]
    out[b,u] = sum_e gw[b,e] * ( (x @ mu[:,:,e])[b,u] + s[b]*se[u,e] + bias[u,e] )
    with s[b] = sum_d x[b,d].

Sharding: data-parallel over batch. Each of the 8 cores processes 512 rows
of x and produces its 512-row slice of the output; the host concatenates.
No collectives needed.

Everything runs through the PE in float32r (the full-rate 4-byte fp32 path;
measured end-to-end relative error vs a float64 reference: 2.4e-4).
"""

import numpy as np
import ml_dtypes

import concourse.bass as bass
from concourse import bacc
import concourse.mybir as mybir
import concourse.tile as tile
from concourse.bass_utils import run_bass_kernel_spmd
from concourse.masks import make_identity

N_CORES = 8
B, D, U, E = 4096, 1024, 1024, 8
P = 128                 # partitions
BS = B // N_CORES       # 512 batch rows per core
KT = D // P             # 8 contraction tiles
BT = BS // P            # 4 batch tiles per core
NT = 512                # matmul moving free dim (one PSUM bank of fp32)
UT = U // NT            # 2 output column tiles

F32 = mybir.dt.float32
F32R = mybir.dt.float32r
BF16 = mybir.dt.bfloat16
AF = mybir.ActivationFunctionType
ALU = mybir.AluOpType

_CACHE: dict = {}


def _emit(nc, tc, xT, muR, gk, gb, rhoT, epsT, biasT, onesd, y):
    with tc.tile_pool(name="const", bufs=1) as cp:
        # Resident inputs
        xt_sb = cp.tile([P, KT, BS], F32R)
        nc.sync.dma_start(out=xt_sb, in_=xT.rearrange("(kt p) b -> p kt b", p=P))
        gk_sb = cp.tile([P, KT, 10], F32R)
        nc.sync.dma_start(out=gk_sb, in_=gk.rearrange("(kt p) e -> p kt e", p=P))
        gb_sb = cp.tile([1, 10], F32R)
        nc.sync.dma_start(out=gb_sb, in_=gb)
        rho_sb = cp.tile([E, U], F32)
        nc.sync.dma_start(out=rho_sb, in_=rhoT)
        eps_sb = cp.tile([E, U], F32)
        nc.sync.dma_start(out=eps_sb, in_=epsT)
        bias_sb = cp.tile([E, U], F32)
        nc.sync.dma_start(out=bias_sb, in_=biasT)

        ones1 = cp.tile([1, P], F32R)
        nc.sync.dma_start(out=ones1, in_=onesd)
        ident = cp.tile([P, P], F32)
        make_identity(nc, ident)

        # Gating/noise intermediates kept for the whole kernel
        gw_sb = cp.tile([P, BT, E], F32)    # renormalized top-2 gates
        s_sb = cp.tile([P, BT], F32)        # per-row sums of x
        gwT_sb = cp.tile([E, BS], F32)      # gates transposed (e on partitions)
        seT_sb = cp.tile([E, U], F32)       # softplus(rho)*eps, (e, u) layout
        c_sb = cp.tile([P, BT, U], F32)     # noise matrix sum_e gw[b,e]*se[u,e]
        c2_sb = cp.tile([P, BT, U], F32)    # bias matrix sum_e gw[b,e]*bias[u,e]

        with (
            tc.tile_pool(name="psum", bufs=1, space="PSUM") as pp,
            tc.tile_pool(name="gtmp", bufs=2) as gt,
            tc.tile_pool(name="wpool", bufs=3) as wp,
            tc.tile_pool(name="ypool", bufs=2) as yp,
        ):
            # ---- noise coefficients se = softplus(rho) * eps ----
            # softplus as ln(1 + exp(rho)); rho ~ -2.6 so exp can't overflow
            nc.scalar.activation(out=seT_sb, in_=rho_sb, func=AF.Exp)
            nc.scalar.activation(out=seT_sb, in_=seT_sb, func=AF.Ln, bias=1.0)
            nc.vector.tensor_mul(seT_sb, seT_sb, eps_sb)

            # ---- gating (per 128-row tile) ----
            for bt in range(BT):
                pg = pp.tile([P, 10], F32, tag="gat", bufs=2)
                for kt in range(KT):
                    nc.tensor.matmul(
                        pg,
                        lhsT=xt_sb[:, kt, bt * P:(bt + 1) * P],
                        rhs=gk_sb[:, kt, :],
                        start=(kt == 0),
                        stop=False,
                    )
                # add gating bias (and 0 for the row-sum column): ones^T x gb_row
                nc.tensor.matmul(pg, lhsT=ones1, rhs=gb_sb, start=False, stop=True)

                logit = pg[:, 0:8]
                m1 = gt.tile([P, 1], F32, tag="m1")
                nc.vector.tensor_reduce(out=m1, in_=logit, axis=mybir.AxisListType.X, op=ALU.max)
                mask = gt.tile([P, 8], F32, tag="mask")
                nc.vector.tensor_scalar(out=mask, in0=logit, scalar1=m1, scalar2=None, op0=ALU.is_equal)
                l2 = gt.tile([P, 8], F32, tag="l2")
                nc.vector.scalar_tensor_tensor(
                    out=l2, in0=mask, scalar=-1e30, in1=logit, op0=ALU.mult, op1=ALU.add
                )
                m2 = gt.tile([P, 1], F32, tag="m2")
                nc.vector.tensor_reduce(out=m2, in_=l2, axis=mybir.AxisListType.X, op=ALU.max)
                nc.vector.tensor_scalar(out=mask, in0=logit, scalar1=m2, scalar2=None, op0=ALU.is_ge)

                el = gt.tile([P, 8], F32, tag="el")
                nc.scalar.activation(out=el, in_=logit, func=AF.Exp)
                gm = gt.tile([P, 8], F32, tag="gm")
                den = gt.tile([P, 1], F32, tag="den")
                nc.vector.scalar_tensor_tensor(
                    out=gm, in0=el, scalar=1.0, in1=mask, op0=ALU.mult, op1=ALU.mult, accum_out=den
                )
                inv = gt.tile([P, 1], F32, tag="inv")
                nc.vector.reciprocal(inv, den)
                nc.vector.tensor_scalar_mul(gw_sb[:, bt, :], gm, inv)
                nc.scalar.copy(s_sb[:, bt:bt + 1], pg[:, 8:9])

                # transpose gates to (e, b) for the noise matmul
                pt = pp.tile([8, P], F32, tag="gat", bufs=2)
                nc.tensor.transpose(pt, gw_sb[:, bt, :], ident)
                nc.scalar.copy(gwT_sb[:, bt * P:(bt + 1) * P], pt)

            # ---- noise matrix c[b,u] = sum_e gw[b,e]*se[u,e]; bias matrix c2 ----
            for bt in range(BT):
                for ut in range(UT):
                    cps = pp.tile([P, NT], F32, tag="gat", bufs=2)
                    nc.tensor.matmul(
                        cps,
                        lhsT=gwT_sb[:, bt * P:(bt + 1) * P],
                        rhs=seT_sb[:, ut * NT:(ut + 1) * NT],
                        start=True,
                        stop=True,
                    )
                    nc.scalar.copy(c_sb[:, bt, ut * NT:(ut + 1) * NT], cps)
                    cps2 = pp.tile([P, NT], F32, tag="gat", bufs=2)
                    nc.tensor.matmul(
                        cps2,
                        lhsT=gwT_sb[:, bt * P:(bt + 1) * P],
                        rhs=bias_sb[:, ut * NT:(ut + 1) * NT],
                        start=True,
                        stop=True,
                    )
                    nc.scalar.copy(c2_sb[:, bt, ut * NT:(ut + 1) * NT], cps2)

            # ---- main expert matmuls + gate-weighted combine ----
            for ut in range(UT):
                ya = []
                for bt in range(BT):
                    t = yp.tile([P, NT], F32, tag=f"ya{bt}", name=f"ya_{ut}_{bt}")
                    ya.append(t)
                for e in range(E):
                    ps = [
                        pp.tile([P, NT], F32, tag="ps", bufs=6, name=f"ps_{ut}_{e}_{bt}")
                        for bt in range(BT)
                    ]
                    w = wp.tile([P, KT, NT], F32R, tag="w", bufs=3, name=f"w_{ut}_{e}")
                    wsrc = muR[e, :, ut * NT:(ut + 1) * NT].rearrange(
                        "(kt p) n -> p kt n", p=P
                    )
                    nc.sync.dma_start(out=w[:, 0:KT // 2, :], in_=wsrc[:, 0:KT // 2, :])
                    nc.sync.dma_start(out=w[:, KT // 2:, :], in_=wsrc[:, KT // 2:, :])
                    for bt in range(BT):
                        for kt in range(KT):
                            nc.tensor.matmul(
                                ps[bt], lhsT=xt_sb[:, kt, bt * P:(bt + 1) * P], rhs=w[:, kt, :],
                                start=(kt == 0), stop=(kt == KT - 1)
                            )
                    for bt in range(BT):
                        gwe = gw_sb[:, bt, e:e + 1]
                        if e == 0:
                            nc.vector.scalar_tensor_tensor(
                                out=ya[bt], in0=ps[bt], scalar=gwe,
                                in1=c2_sb[:, bt, ut * NT:(ut + 1) * NT],
                                op0=ALU.mult, op1=ALU.add,
                            )
                        else:
                            nc.vector.scalar_tensor_tensor(
                                out=ya[bt], in0=ps[bt], scalar=gwe, in1=ya[bt],
                                op0=ALU.mult, op1=ALU.add,
                            )
                # add noise term and store
                for bt in range(BT):
                    yo = yp.tile([P, NT], F32, tag="yo", bufs=3, name=f"yo_{ut}_{bt}")
                    nc.vector.scalar_tensor_tensor(
                        out=yo, in0=c_sb[:, bt, ut * NT:(ut + 1) * NT],
                        scalar=s_sb[:, bt:bt + 1], in1=ya[bt],
                        op0=ALU.mult, op1=ALU.add,
                    )
                    nc.sync.dma_start(out=y[bt * P:(bt + 1) * P, ut * NT:(ut + 1) * NT], in_=yo)


def build(reps=1):
    key = ("nc", reps)
    if key in _CACHE:
        return _CACHE[key]
    nc = bacc.Bacc("TRN2", target_bir_lowering=False)
    xT = nc.dram_tensor("xT", [D, BS], F32R, kind="ExternalInput").ap()
    muR = nc.dram_tensor("muR", [E, D, U], F32R, kind="ExternalInput").ap()
    gk = nc.dram_tensor("gk", [D, 10], F32R, kind="ExternalInput").ap()
    gb = nc.dram_tensor("gb", [1, 10], F32R, kind="ExternalInput").ap()
    rhoT = nc.dram_tensor("rhoT", [E, U], F32, kind="ExternalInput").ap()
    biasT = nc.dram_tensor("biasT", [E, U], F32, kind="ExternalInput").ap()
    epsT = nc.dram_tensor("epsT", [E, U], F32, kind="ExternalInput").ap()
    onesd = nc.dram_tensor("onesd", [1, P], F32R, kind="ExternalInput").ap()
    y = nc.dram_tensor("y", [BS, U], F32, kind="ExternalOutput").ap()
    with tile.TileContext(nc) as tc:
        if reps == 1:
            _emit(nc, tc, xT, muR, gk, gb, rhoT, epsT, biasT, onesd, y)
        else:
            with tc.For_i(0, reps, 1):
                _emit(nc, tc, xT, muR, gk, gb, rhoT, epsT, biasT, onesd, y)
    nc.compile()
    _CACHE[key] = nc
    return nc


def prep_inputs(x, expert_mu, expert_rho, expert_bias, gating_kernel, gating_bias, eps):
    """Host-side sharding / layout prep (no math beyond dtype rounding)."""
    x = np.ascontiguousarray(np.asarray(x, dtype=np.float32))
    mu = np.asarray(expert_mu, dtype=np.float32)        # [D, U, E]
    bias = np.asarray(expert_bias, dtype=np.float32)    # [U, E]
    # e-major weights, bf16 for the main matmul
    muR = np.ascontiguousarray(np.transpose(mu, (2, 0, 1)))
    gk = np.concatenate(
        [np.asarray(gating_kernel, dtype=np.float32), np.ones((D, 1), np.float32),
         np.zeros((D, 1), np.float32)], axis=1
    )  # [D, 10]: col 8 computes the row-sums s; col 9 pads to even width (fp32r ISA)
    gb = np.concatenate(
        [np.asarray(gating_bias, dtype=np.float32), np.zeros((2,), np.float32)]
    ).reshape(1, 10)
    rhoT = np.ascontiguousarray(np.asarray(expert_rho, dtype=np.float32).T)  # [E, U]
    epsT = np.ascontiguousarray(np.asarray(eps, dtype=np.float32).T)         # [E, U]
    biasT = np.ascontiguousarray(bias.T)                                     # [E, U]
    shared = {"muR": muR, "gk": gk, "gb": gb, "rhoT": rhoT, "epsT": epsT, "biasT": biasT,
              "onesd": np.ones((1, P), np.float32)}
    in_maps = []
    for c in range(N_CORES):
        xs = np.ascontiguousarray(x[c * BS:(c + 1) * BS].T)  # [D, BS]
        in_maps.append({"xT": xs, **shared})
    return in_maps


def kernel(x, expert_mu, expert_rho, expert_bias, gating_kernel, gating_bias, eps, k):
    assert int(k) == 2, f"kernel is specialized for top-2 gating, got k={k}"
    nc = build()
    in_maps = prep_inputs(
        x, expert_mu, expert_rho, expert_bias, gating_kernel, gating_bias, eps
    )
    res = run_bass_kernel_spmd(nc, in_maps, list(range(N_CORES)))
    return np.concatenate([res.results[c]["y"] for c in range(N_CORES)], axis=0)



# revision 3
# speedup vs baseline: 7.2598x; 7.2598x over previous
"""Bayesian dense MoE (top-2 of 8 experts) on 8 Trainium2 NeuronCores.

Math (per reference):
    logits = x @ gk + gb                      [B, E]
    gw     = renorm-top2(softmax(logits))     [B, E]   (softmax denom cancels)
    se     = softplus(rho) * eps              [U, E]
    out[b,u] = sum_e gw[b,e] * ( (x @ mu[:,:,e])[b,u] + s[b]*se[u,e] + bias[u,e] )
    with s[b] = sum_d x[b,d].

Sharding: data-parallel over batch. Each of the 8 cores processes 512 rows
of x and produces its 512-row slice of the output; the host concatenates.
No collectives needed.

Key optimizations over the fp32r streaming version:
  - Expert weights mu are scaled by 512, cast to fp8e4 (TRN E4M3, max 240)
    on the host, and kept RESIDENT in SBUF (8 MB) — loaded once outside the
    iteration loop, so steady-state HBM traffic is just x in / y out.
  - The mean-path matmuls run in fp8 DoubleRow perf mode (2 contraction
    subtiles per instruction, 2x PE throughput). The 1/512 de-scale is
    folded into the gate weights used by the combine.
  - Gating stays in fp32r on an exact fp32 copy of x, so top-2 selection
    matches the reference bit-for-bit on these inputs.
  - Bodies are emitted twice per hardware-loop trip with double-buffered
    input tiles, so iteration i+1's x DMA overlaps iteration i's compute.

Measured end-to-end relative error vs a float64 reference: ~8e-3
(fp8 quantization of x and mu; tolerance is 2e-2).
"""

import numpy as np
import ml_dtypes

import concourse.bass as bass
from concourse import bacc
import concourse.mybir as mybir
import concourse.tile as tile
from concourse.bass_utils import run_bass_kernel_spmd
from concourse.masks import make_identity

N_CORES = 8
B, D, U, E = 4096, 1024, 1024, 8
P = 128                 # partitions
BS = B // N_CORES       # 512 batch rows per core
KT = D // P             # 8 contraction subtiles
KP = KT // 2            # 4 DoubleRow contraction pairs
BT = BS // P            # 4 batch tiles per core
NT = 512                # matmul moving free dim (one PSUM bank of fp32)
UT = U // NT            # 2 output column tiles
MU_SCALE = 512.0        # fp8 range scaling for mu (|mu*512| < 40 << 240)

F32 = mybir.dt.float32
F32R = mybir.dt.float32r
F8 = mybir.dt.float8e4
BF16 = mybir.dt.bfloat16
F8NP = ml_dtypes.float8_e4m3
AF = mybir.ActivationFunctionType
ALU = mybir.AluOpType
DR = mybir.MatmulPerfMode.DoubleRow

_CACHE: dict = {}


def _consts(nc, tc, cp, t):
    """Load loop-invariant tensors into SBUF and precompute se = softplus(rho)*eps."""
    C = {}
    C["mu8"] = cp.tile([P, E, KT, U], F8, name="mu8")
    nc.sync.dma_start(
        out=C["mu8"], in_=t["muR8"].rearrange("e (kt p) u -> p e kt u", p=P)
    )
    C["gk"] = cp.tile([P, KT, 10], F32R, name="gk_sb")
    nc.sync.dma_start(out=C["gk"], in_=t["gk"].rearrange("(kt p) e -> p kt e", p=P))
    C["gb"] = cp.tile([1, 10], F32R, name="gb_sb")
    nc.sync.dma_start(out=C["gb"], in_=t["gb"])
    rho = cp.tile([E, U], F32)
    nc.sync.dma_start(out=rho, in_=t["rhoT"])
    eps = cp.tile([E, U], F32)
    nc.sync.dma_start(out=eps, in_=t["epsT"])
    C["bias"] = cp.tile([E, U], F32, name="bias_sb")
    nc.sync.dma_start(out=C["bias"], in_=t["biasT"])
    C["ones1"] = cp.tile([1, P], F32R, name="ones1")
    nc.sync.dma_start(out=C["ones1"], in_=t["onesd"])
    C["ident"] = cp.tile([P, P], F32, name="ident")
    make_identity(nc, C["ident"])

    # se = softplus(rho) * eps, as ln(1 + exp(rho)); rho ~ -2.6 so no overflow
    C["seT"] = cp.tile([E, U], F32, name="seT")
    nc.scalar.activation(out=C["seT"], in_=rho, func=AF.Exp)
    nc.scalar.activation(out=C["seT"], in_=C["seT"], func=AF.Ln, bias=1.0)
    nc.vector.tensor_mul(C["seT"], C["seT"], eps)
    return C


def _body(nc, tc, C, ip, pp, t):
    """One full inference: x (HBM) -> y (HBM)."""
    xt = ip.tile([P, KT, BS], F32R, tag="xt", bufs=2)
    nc.sync.dma_start(out=xt, in_=t["xT"].rearrange("(kt p) b -> p kt b", p=P))
    x8 = ip.tile([P, KT, BS], F8, tag="x8", bufs=2)
    nc.sync.dma_start(out=x8, in_=t["x8T"].rearrange("(kt p) b -> p kt b", p=P))

    gw = ip.tile([P, BT, E], F32, tag="gw", bufs=2)    # renormalized top-2 gates
    gws = ip.tile([P, BT, E], F32, tag="gws", bufs=2)  # gw / MU_SCALE
    s = ip.tile([P, BT], F32, tag="s", bufs=2)         # per-row sums of x
    gwT = ip.tile([E, BS], F32, tag="gwT", bufs=2)     # gates transposed
    c = ip.tile([P, BT, U], BF16, tag="c", bufs=2)     # sum_e gw[b,e]*se[u,e]
    c2 = ip.tile([P, BT, U], BF16, tag="c2", bufs=2)   # sum_e gw[b,e]*bias[u,e]

    # ---- gating (per 128-row tile) ----
    for bt in range(BT):
        pg = pp.tile([P, 10], F32, tag="gat", bufs=2)
        for kt in range(KT):
            nc.tensor.matmul(
                pg,
                lhsT=xt[:, kt, bt * P:(bt + 1) * P],
                rhs=C["gk"][:, kt, :],
                start=(kt == 0),
                stop=False,
            )
        # add gating bias (and 0 for the row-sum column): ones^T x gb_row
        nc.tensor.matmul(pg, lhsT=C["ones1"], rhs=C["gb"], start=False, stop=True)

        logit = pg[:, 0:8]
        m1 = ip.tile([P, 1], F32, tag="m1", bufs=2)
        nc.vector.tensor_reduce(out=m1, in_=logit, axis=mybir.AxisListType.X, op=ALU.max)
        mask = ip.tile([P, 8], F32, tag="mask", bufs=2)
        nc.vector.tensor_scalar(out=mask, in0=logit, scalar1=m1, scalar2=None, op0=ALU.is_equal)
        l2 = ip.tile([P, 8], F32, tag="l2", bufs=2)
        nc.vector.scalar_tensor_tensor(
            out=l2, in0=mask, scalar=-1e30, in1=logit, op0=ALU.mult, op1=ALU.add
        )
        m2 = ip.tile([P, 1], F32, tag="m2", bufs=2)
        nc.vector.tensor_reduce(out=m2, in_=l2, axis=mybir.AxisListType.X, op=ALU.max)
        nc.vector.tensor_scalar(out=mask, in0=logit, scalar1=m2, scalar2=None, op0=ALU.is_ge)

        el = ip.tile([P, 8], F32, tag="el", bufs=2)
        nc.scalar.activation(out=el, in_=logit, func=AF.Exp)
        gm = ip.tile([P, 8], F32, tag="gm", bufs=2)
        den = ip.tile([P, 1], F32, tag="den", bufs=2)
        nc.vector.scalar_tensor_tensor(
            out=gm, in0=el, scalar=1.0, in1=mask, op0=ALU.mult, op1=ALU.mult, accum_out=den
        )
        inv = ip.tile([P, 1], F32, tag="inv", bufs=2)
        nc.vector.reciprocal(inv, den)
        nc.vector.tensor_scalar_mul(gw[:, bt, :], gm, inv)
        nc.vector.tensor_scalar(
            out=gws[:, bt, :], in0=gw[:, bt, :], scalar1=1.0 / MU_SCALE, scalar2=None,
            op0=ALU.mult,
        )
        nc.scalar.copy(s[:, bt:bt + 1], pg[:, 8:9])

        # transpose gates to (e, b) for the noise/bias matmuls
        pt = pp.tile([8, P], F32, tag="gat", bufs=2)
        nc.tensor.transpose(pt, gw[:, bt, :], C["ident"])
        nc.scalar.copy(gwT[:, bt * P:(bt + 1) * P], pt)

    # ---- noise matrix c[b,u] = sum_e gw[b,e]*se[u,e]; bias matrix c2 ----
    for bt in range(BT):
        for ut in range(UT):
            cps = pp.tile([P, NT], F32, tag="cps", bufs=2)
            nc.tensor.matmul(
                cps,
                lhsT=gwT[:, bt * P:(bt + 1) * P],
                rhs=C["seT"][:, ut * NT:(ut + 1) * NT],
                start=True,
                stop=True,
            )
            nc.scalar.copy(c[:, bt, ut * NT:(ut + 1) * NT], cps)
            cps2 = pp.tile([P, NT], F32, tag="cps", bufs=2)
            nc.tensor.matmul(
                cps2,
                lhsT=gwT[:, bt * P:(bt + 1) * P],
                rhs=C["bias"][:, ut * NT:(ut + 1) * NT],
                start=True,
                stop=True,
            )
            nc.scalar.copy(c2[:, bt, ut * NT:(ut + 1) * NT], cps2)

    # ---- fp8 DoubleRow expert matmuls + gate-weighted combine ----
    for ut in range(UT):
        for bt in range(BT):
            ya = ip.tile([P, NT], F32, tag="ya", bufs=2)
            for e in range(E):
                ps = pp.tile([P, NT], F32, tag="ps", bufs=4)
                for kp in range(KP):
                    nc.tensor.matmul(
                        ps,
                        lhsT=x8[:, 2 * kp:2 * kp + 2, bt * P:(bt + 1) * P],
                        rhs=C["mu8"][:, e, 2 * kp:2 * kp + 2, ut * NT:(ut + 1) * NT],
                        start=(kp == 0),
                        stop=(kp == KP - 1),
                        perf_mode=DR,
                    )
                gwe = gws[:, bt, e:e + 1]
                if e == 0:
                    nc.vector.scalar_tensor_tensor(
                        out=ya, in0=ps, scalar=gwe,
                        in1=c2[:, bt, ut * NT:(ut + 1) * NT],
                        op0=ALU.mult, op1=ALU.add,
                    )
                else:
                    nc.vector.scalar_tensor_tensor(
                        out=ya, in0=ps, scalar=gwe, in1=ya,
                        op0=ALU.mult, op1=ALU.add,
                    )
            # add noise term and store
            yo = ip.tile([P, NT], F32, tag="yo", bufs=3)
            nc.vector.scalar_tensor_tensor(
                out=yo, in0=c[:, bt, ut * NT:(ut + 1) * NT],
                scalar=s[:, bt:bt + 1], in1=ya,
                op0=ALU.mult, op1=ALU.add,
            )
            nc.sync.dma_start(
                out=t["y"][bt * P:(bt + 1) * P, ut * NT:(ut + 1) * NT], in_=yo
            )


def build(reps=1):
    key = ("nc", reps)
    if key in _CACHE:
        return _CACHE[key]
    nc = bacc.Bacc("TRN2", target_bir_lowering=False)
    t = {
        "xT": nc.dram_tensor("xT", [D, BS], F32R, kind="ExternalInput").ap(),
        "x8T": nc.dram_tensor("x8T", [D, BS], F8, kind="ExternalInput").ap(),
        "muR8": nc.dram_tensor("muR8", [E, D, U], F8, kind="ExternalInput").ap(),
        "gk": nc.dram_tensor("gk", [D, 10], F32R, kind="ExternalInput").ap(),
        "gb": nc.dram_tensor("gb", [1, 10], F32R, kind="ExternalInput").ap(),
        "rhoT": nc.dram_tensor("rhoT", [E, U], F32, kind="ExternalInput").ap(),
        "biasT": nc.dram_tensor("biasT", [E, U], F32, kind="ExternalInput").ap(),
        "epsT": nc.dram_tensor("epsT", [E, U], F32, kind="ExternalInput").ap(),
        "onesd": nc.dram_tensor("onesd", [1, P], F32R, kind="ExternalInput").ap(),
        "y": nc.dram_tensor("y", [BS, U], F32, kind="ExternalOutput").ap(),
    }
    with tile.TileContext(nc) as tc:
        with tc.tile_pool(name="const", bufs=1) as cp:
            C = _consts(nc, tc, cp, t)
            with (
                tc.tile_pool(name="iter", bufs=1) as ip,
                tc.tile_pool(name="psum", bufs=1, space="PSUM") as pp,
            ):
                if reps == 1:
                    _body(nc, tc, C, ip, pp, t)
                else:
                    assert reps % 2 == 0, "loop body is emitted twice per trip"
                    with tc.For_i(0, reps // 2, 1):
                        _body(nc, tc, C, ip, pp, t)
                        _body(nc, tc, C, ip, pp, t)
    nc.compile()
    _CACHE[key] = nc
    return nc


def prep_inputs(x, expert_mu, expert_rho, expert_bias, gating_kernel, gating_bias, eps):
    """Host-side sharding / layout prep (transpose, dtype cast, replication)."""
    x = np.ascontiguousarray(np.asarray(x, dtype=np.float32))
    mu = np.asarray(expert_mu, dtype=np.float32)        # [D, U, E]
    # e-major fp8 weights, scaled into the TRN E4M3 normal range
    muR8 = np.ascontiguousarray(
        np.clip(np.transpose(mu, (2, 0, 1)) * MU_SCALE, -240.0, 240.0)
    ).astype(F8NP)
    gk = np.concatenate(
        [np.asarray(gating_kernel, dtype=np.float32), np.ones((D, 1), np.float32),
         np.zeros((D, 1), np.float32)], axis=1
    )  # [D, 10]: col 8 computes the row-sums s; col 9 pads to even width (fp32r ISA)
    gb = np.concatenate(
        [np.asarray(gating_bias, dtype=np.float32), np.zeros((2,), np.float32)]
    ).reshape(1, 10)
    rhoT = np.ascontiguousarray(np.asarray(expert_rho, dtype=np.float32).T)  # [E, U]
    epsT = np.ascontiguousarray(np.asarray(eps, dtype=np.float32).T)         # [E, U]
    biasT = np.ascontiguousarray(np.asarray(expert_bias, dtype=np.float32).T)
    shared = {"muR8": muR8, "gk": gk, "gb": gb, "rhoT": rhoT, "epsT": epsT,
              "biasT": biasT, "onesd": np.ones((1, P), np.float32)}
    in_maps = []
    for cid in range(N_CORES):
        xs = np.ascontiguousarray(x[cid * BS:(cid + 1) * BS].T)  # [D, BS]
        in_maps.append({"xT": xs, "x8T": xs.astype(F8NP), **shared})
    return in_maps


def kernel(x, expert_mu, expert_rho, expert_bias, gating_kernel, gating_bias, eps, k):
    assert int(k) == 2, f"kernel is specialized for top-2 gating, got k={k}"
    nc = build()
    in_maps = prep_inputs(
        x, expert_mu, expert_rho, expert_bias, gating_kernel, gating_bias, eps
    )
    res = run_bass_kernel_spmd(nc, in_maps, list(range(N_CORES)))
    return np.concatenate([res.results[c]["y"] for c in range(N_CORES)], axis=0)
